# revision 5
# baseline (speedup 1.0000x reference)
"""Trainium2 Bass kernel for nn_CrossAttention (B=2, N=2048, C=1024, H=16).

Sharding: 16 heads / 8 cores = 2 heads per core (both batches on every
core).  Each core computes its heads' Q/K/V projections with the matching
128-row slice of Wq/Wk/Wv, full attention for its 4 (batch, head) pairs,
and a partial output projection against its 128-column slice of Wp.  The
host sums the 8 partial projections (the tensor-parallel all-reduce) and
adds the bias.

On-device layouts (per core, fp16 matmul operands / fp32 PSUM):
  xq/xt   [1024 ch, 4096 pos]   channel-major inputs (host pre-transposed)
  q2T/k2T/v2T [128, 4096]       rows 0-63 head0 dims, 64-127 head1 dims
  vpos    [128 keys, 4x16x64]   v transposed back to key-major via PE
  S_T     [128 keys, 2048 q]    exp(scores^T) per (head, key-block), fp16
  outTb   [128 dims, 2048 q]    normalized attention output, per batch
  out_p   [2, 2048, 1024] f32   partial projection (summed on host)
"""

import os
import sys

for _p in ("/opt/trn_rl_repo", os.path.expanduser("~/.axon_site/_ro/trn_rl_repo")):
    if os.path.isdir(_p) and _p not in sys.path:
        sys.path.insert(0, _p)

import numpy as np

import concourse.bacc as bacc
import concourse.mybir as mybir
import concourse.tile as tile
from concourse.bass_utils import run_bass_kernel_spmd

F16 = mybir.dt.float16
F32 = mybir.dt.float32
AF = mybir.ActivationFunctionType

B, N, C, H, D = 2, 2048, 1024, 16, 64
NCORES = 8
SCALE = float(D) ** -0.5

# Set by test harness to collect an NTFF profile.
TRACE = False
LAST_EXEC_NS = None
LAST_RESULTS = None

_COMPILED_NC = None


def _emit(nc):
    xq = nc.dram_tensor("xq", [C, B * N], F16, kind="ExternalInput")
    xt = nc.dram_tensor("xt", [C, B * N], F16, kind="ExternalInput")
    wq = nc.dram_tensor("wq", [8, 128, 128], F16, kind="ExternalInput")
    wk = nc.dram_tensor("wk", [8, 128, 128], F16, kind="ExternalInput")
    wv = nc.dram_tensor("wv", [8, 128, 128], F16, kind="ExternalInput")
    wp = nc.dram_tensor("wp", [128, C], F16, kind="ExternalInput")
    id64 = nc.dram_tensor("id64", [128, 64], F16, kind="ExternalInput")
    outp = nc.dram_tensor("out_p", [B, N, C], F32, kind="ExternalOutput")

    with tile.TileContext(nc) as tc:
        with (
            tc.tile_pool(name="consts", bufs=1) as cpool,
            tc.tile_pool(name="xs", bufs=3) as xs,
            tc.tile_pool(name="big", bufs=1) as big,
            tc.tile_pool(name="st", bufs=32) as stp,
            tc.tile_pool(name="ob", bufs=1) as obp,
            tc.tile_pool(name="rc", bufs=2) as rcpool,
            tc.tile_pool(name="pe", bufs=3) as pep,
            tc.tile_pool(name="ps", bufs=2, space="PSUM") as psp,
        ):
            # ---- constants -------------------------------------------------
            w_sb = {}
            for name, dram in (("wq", wq), ("wk", wk), ("wv", wv)):
                t = cpool.tile([128, 1024], F16, tag=name)
                for kc in range(8):
                    nc.sync.dma_start(
                        t[:, kc * 128 : (kc + 1) * 128], dram[kc, :, :]
                    )
                w_sb[name] = t
            wp_sb = cpool.tile([128, C], F16, tag="wp")
            nc.sync.dma_start(wp_sb[:], wp[:])
            id_sb = cpool.tile([128, 64], F16, tag="id64")
            nc.sync.dma_start(id_sb[:], id64[:])
            ones_sb = cpool.tile([128, 1], F16, tag="ones")
            nc.vector.memset(ones_sb[:], 1.0)

            q2T = big.tile([128, B * N], F16, tag="q2T")
            k2T = big.tile([128, B * N], F16, tag="k2T")
            v2T = big.tile([128, B * N], F16, tag="v2T")
            vpos = big.tile([128, B * 2 * N], F16, tag="vpos")

            # ---- Q projection: q2T[:, half] = (Wq_c^T)^T @ xq -------------
            for half in range(2):
                hs = slice(half * 2048, (half + 1) * 2048)
                ps_q = psp.tile([128, 2048], F32, tag="ps")
                for kc in range(8):
                    xq_t = xs.tile([128, 2048], F16, tag="x")
                    nc.sync.dma_start(
                        xq_t[:], xq[kc * 128 : (kc + 1) * 128, hs]
                    )
                    for qc in range(4):
                        cs = slice(qc * 512, (qc + 1) * 512)
                        nc.tensor.matmul(
                            ps_q[:, cs],
                            lhsT=w_sb["wq"][:, kc * 128 : (kc + 1) * 128],
                            rhs=xq_t[:, cs],
                            start=(kc == 0),
                            stop=(kc == 7),
                        )
                nc.vector.tensor_copy(q2T[:, hs], ps_q[:])

            # ---- K and V projections share the xt stream ------------------
            for half in range(2):
                hs = slice(half * 2048, (half + 1) * 2048)
                ps_k = psp.tile([128, 2048], F32, tag="ps")
                ps_v = psp.tile([128, 2048], F32, tag="ps")
                for kc in range(8):
                    xt_t = xs.tile([128, 2048], F16, tag="x")
                    nc.sync.dma_start(
                        xt_t[:], xt[kc * 128 : (kc + 1) * 128, hs]
                    )
                    for qc in range(4):
                        cs = slice(qc * 512, (qc + 1) * 512)
                        nc.tensor.matmul(
                            ps_k[:, cs],
                            lhsT=w_sb["wk"][:, kc * 128 : (kc + 1) * 128],
                            rhs=xt_t[:, cs],
                            start=(kc == 0),
                            stop=(kc == 7),
                        )
                        nc.tensor.matmul(
                            ps_v[:, cs],
                            lhsT=w_sb["wv"][:, kc * 128 : (kc + 1) * 128],
                            rhs=xt_t[:, cs],
                            start=(kc == 0),
                            stop=(kc == 7),
                        )
                nc.vector.tensor_copy(k2T[:, hs], ps_k[:])
                nc.vector.tensor_copy(v2T[:, hs], ps_v[:])

            # ---- transpose v2T -> vpos (key-major) via PE -----------------
            for b in range(2):
                for h in range(2):
                    hp = slice(h * 64, (h + 1) * 64)
                    for oct_ in range(2):
                        ps_t = psp.tile([128, 512], F16, tag="ps")
                        for i in range(8):
                            m = oct_ * 8 + i
                            ks = slice(b * 2048 + m * 128, b * 2048 + (m + 1) * 128)
                            nc.tensor.transpose(
                                ps_t[:, i * 64 : (i + 1) * 64],
                                v2T[hp, ks],
                                id_sb[hp, :],
                            )
                        nc.vector.tensor_copy(
                            vpos[
                                :,
                                (b * 2 + h) * 1024
                                + oct_ * 512 : (b * 2 + h) * 1024
                                + (oct_ + 1) * 512,
                            ],
                            ps_t[:],
                        )

            # ---- per-batch attention + projection -------------------------
            for b in range(2):
                bbase = b * 2048
                # scores^T -> exp -> S_T tiles (fp16)
                st_tiles = {}
                for m in range(16):
                    ms = slice(bbase + m * 128, bbase + (m + 1) * 128)
                    ps_s = [
                        psp.tile([128, 2048], F32, tag="ps", name=f"ps_s{h}")
                        for h in range(2)
                    ]
                    for qc in range(4):
                        cs = slice(qc * 512, (qc + 1) * 512)
                        qs = slice(bbase + qc * 512, bbase + (qc + 1) * 512)
                        for h in range(2):
                            hp = slice(h * 64, (h + 1) * 64)
                            nc.tensor.matmul(
                                ps_s[h][:, cs],
                                lhsT=k2T[hp, ms],
                                rhs=q2T[hp, qs],
                                start=True,
                                stop=True,
                            )
                    for h in range(2):
                        st = stp.tile([128, 2048], F16, tag="st")
                        nc.scalar.activation(st[:], ps_s[h][:], AF.Exp, scale=SCALE)
                        st_tiles[(h, m)] = st

                # attnv (col-tiled head pair) + denominators (ones matmuls)
                ps_o = psp.tile([128, 2048], F32, tag="ps")
                ps_d = psp.tile([128, 2048], F32, tag="ps")
                for m in range(16):
                    for qc in range(4):
                        cs = slice(qc * 512, (qc + 1) * 512)
                        for h in range(2):
                            nc.tensor.matmul(
                                ps_o[h * 64 : (h + 1) * 64, cs],
                                lhsT=vpos[
                                    :,
                                    (b * 2 + h) * 1024
                                    + m * 64 : (b * 2 + h) * 1024
                                    + (m + 1) * 64,
                                ],
                                rhs=st_tiles[(h, m)][:, cs],
                                start=(m == 0),
                                stop=(m == 15),
                            )
                    for qc in range(4):
                        cs = slice(qc * 512, (qc + 1) * 512)
                        for h, prow in ((0, 0), (1, 32)):
                            nc.tensor.matmul(
                                ps_d[prow : prow + 1, cs],
                                lhsT=ones_sb[:, 0:1],
                                rhs=st_tiles[(h, m)][:, cs],
                                start=(m == 0),
                                stop=(m == 15),
                            )

                # normalize: outTb = ps_o * (1/denom broadcast)
                outTb = obp.tile([128, 2048], F16, tag="outT")
                for qc in range(4):
                    cs = slice(qc * 512, (qc + 1) * 512)
                    rc = rcpool.tile([128, 512], F32, tag="rc")
                    nc.vector.reciprocal(rc[0:1, :], ps_d[0:1, cs])
                    nc.vector.reciprocal(rc[32:33, :], ps_d[32:33, cs])
                    bcast = [0] * 32
                    # h1 denom (row 32) -> rows 64..127, then h0 (row 0)
                    # -> rows 0..63 (row 32 consumed before overwrite).
                    nc.vector.stream_shuffle(rc[64:96, :], rc[32:64, :], bcast)
                    nc.vector.stream_shuffle(rc[96:128, :], rc[32:64, :], bcast)
                    nc.vector.stream_shuffle(rc[32:64, :], rc[0:32, :], bcast)
                    nc.vector.stream_shuffle(rc[0:32, :], rc[0:32, :], bcast)
                    nc.vector.tensor_mul(outTb[:, cs], ps_o[:, cs], rc[:])

                # output projection: out_p[b] = outTb^T @ wp_c^T (partial)
                for pm in range(16):
                    ps_p = psp.tile([128, 1024], F32, tag="ps")
                    for ncol in range(2):
                        nc.tensor.matmul(
                            ps_p[:, ncol * 512 : (ncol + 1) * 512],
                            lhsT=outTb[:, pm * 128 : (pm + 1) * 128],
                            rhs=wp_sb[:, ncol * 512 : (ncol + 1) * 512],
                            start=True,
                            stop=True,
                        )
                    for ncol in range(2):
                        ev = pep.tile([128, 512], F32, tag="pe")
                        nc.vector.tensor_copy(
                            ev[:], ps_p[:, ncol * 512 : (ncol + 1) * 512]
                        )
                        nc.sync.dma_start(
                            outp[
                                b,
                                pm * 128 : (pm + 1) * 128,
                                ncol * 512 : (ncol + 1) * 512,
                            ],
                            ev[:],
                        )
    return nc


def _get_compiled():
    global _COMPILED_NC
    if _COMPILED_NC is None:
        nc = bacc.Bacc(
            "TRN2", target_bir_lowering=False, debug=False, num_devices=NCORES
        )
        _emit(nc)
        nc.compile()
        _COMPILED_NC = nc
    return _COMPILED_NC


def _install_trace_shim():
    """Register antenv.axon_hooks NTFF hook (missing on this image)."""
    import contextlib
    import ctypes
    import types

    if "antenv.axon_hooks" in sys.modules:
        return
    try:
        import antenv
    except ImportError:
        return
    so_path = "/opt/axon/libaxon_pjrt.so"
    if not os.path.exists(so_path):
        return

    mod = types.ModuleType("antenv.axon_hooks")
    mod._hook = None
    mod.set_axon_ntff_profile_hook = lambda h: setattr(mod, "_hook", h)
    mod.get_axon_ntff_profile_hook = lambda: mod._hook

    lib = ctypes.CDLL(so_path)
    if not hasattr(lib, "axon_start_nrt_profile"):
        return
    lib.axon_start_nrt_profile.argtypes = [
        ctypes.POINTER(ctypes.c_int64),
        ctypes.c_size_t,
    ]
    lib.axon_start_nrt_profile.restype = ctypes.c_int64
    lib.axon_stop_nrt_profile.argtypes = [ctypes.c_char_p]
    lib.axon_stop_nrt_profile.restype = ctypes.c_int64

    @contextlib.contextmanager
    def _hook(output_dir, device_ids):
        import jax

        jax.devices()
        if device_ids:
            ids = (ctypes.c_int64 * len(device_ids))(*device_ids)
            rc = lib.axon_start_nrt_profile(ids, len(device_ids))
        else:
            rc = lib.axon_start_nrt_profile(None, 0)
        if rc != 0:
            raise RuntimeError(f"axon_start_nrt_profile rc={rc}")
        try:
            yield
        finally:
            n = lib.axon_stop_nrt_profile(str(output_dir).encode())
            if n < 0:
                raise RuntimeError(f"axon_stop_nrt_profile rc={n}")

    mod.set_axon_ntff_profile_hook(_hook)
    sys.modules["antenv.axon_hooks"] = mod
    antenv.axon_hooks = mod


def kernel(query, target, Wq, Wk, Wv, Wp, bp):
    global LAST_EXEC_NS, LAST_RESULTS
    query = np.asarray(query, dtype=np.float32)
    target = np.asarray(target, dtype=np.float32)
    Wq = np.asarray(Wq, dtype=np.float32)
    Wk = np.asarray(Wk, dtype=np.float32)
    Wv = np.asarray(Wv, dtype=np.float32)
    Wp = np.asarray(Wp, dtype=np.float32)
    bp = np.asarray(bp, dtype=np.float32)

    xq = np.ascontiguousarray(query.reshape(B * N, C).T).astype(np.float16)
    xt = np.ascontiguousarray(target.reshape(B * N, C).T).astype(np.float16)
    id64 = np.zeros((128, 64), dtype=np.float16)
    for p in range(128):
        id64[p, p % 64] = 1.0

    in_maps = []
    for c in range(NCORES):
        rows = slice(c * 128, (c + 1) * 128)
        in_maps.append(
            {
                "xq": xq,
                "xt": xt,
                "wq": np.ascontiguousarray(Wq[rows, :].T)
                .astype(np.float16)
                .reshape(8, 128, 128),
                "wk": np.ascontiguousarray(Wk[rows, :].T)
                .astype(np.float16)
                .reshape(8, 128, 128),
                "wv": np.ascontiguousarray(Wv[rows, :].T)
                .astype(np.float16)
                .reshape(8, 128, 128),
                "wp": np.ascontiguousarray(Wp[:, rows].T).astype(np.float16),
                "id64": id64,
            }
        )

    if TRACE:
        _install_trace_shim()

    nc = _get_compiled()
    res = run_bass_kernel_spmd(
        nc, in_maps, core_ids=list(range(NCORES)), trace=TRACE
    )
    LAST_RESULTS = res
    LAST_EXEC_NS = res.exec_time_ns

    acc = res.results[0]["out_p"].astype(np.float64)
    for c in range(1, NCORES):
        acc += res.results[c]["out_p"]
    out = acc.astype(np.float32) + bp[None, None, :]
    return out


# revision 12
# speedup vs baseline: 1.1835x; 1.1835x over previous
"""Trainium2 Bass kernel for nn_CrossAttention (B=2, N=2048, C=1024, H=16).

Sharding: 16 heads / 8 cores = 2 heads per core (both batches on every
core).  Each core computes its heads' Q/K/V projections with the matching
128-row slice of Wq/Wk/Wv, full attention for its 4 (batch, head) pairs,
and a partial output projection against its 128-column slice of Wp.  The
host sums the 8 partial projections (the tensor-parallel all-reduce) and
adds the bias.

Schedule (per core): the exp of 16.8M score elements on the Scalar engine
(~134us) is the critical resource, so scores->exp for batch b+/-0 is
software-pipelined against attnv/denominator matmuls at lag-1 so the
Scalar engine never starves and the PE never idles long enough for HAM to
re-throttle its clock.

PSUM budget (8 banks of 2KB/partition):
  sc pool   2 x [128,1024] f32 = 4 banks  (score regions / qkv / proj)
  o  pool   1 x [128,1024] f32 = 2 banks  (attnv accumulator per qp pass)
  d  pool   1 x [128,1024] f32 = 2 banks  (denominator rows, packed)

On-device layouts (per core, fp16 matmul operands / fp32 PSUM):
  xq/xt   [1024 ch, 4096 pos]   channel-major inputs (host pre-transposed)
  q2T/k2T/v2T [128, 4096]       rows 0-63 head0 dims, 64-127 head1 dims
  vpos    [128 keys, 4x16x64]   v transposed back to key-major via PE
  S_T     [128 keys, 1024 q]    exp(scores^T) tile per (h, m, qp), fp16
  outTb   [128 dims, 2048 q]    normalized attention output, per batch
  out_p   [2, 2048, 1024] f32   partial projection (summed on host)
"""

import os
import sys

for _p in ("/opt/trn_rl_repo", os.path.expanduser("~/.axon_site/_ro/trn_rl_repo")):
    if os.path.isdir(_p) and _p not in sys.path:
        sys.path.insert(0, _p)

import numpy as np

import concourse.bacc as bacc
import concourse.mybir as mybir
import concourse.tile as tile
from concourse.bass_utils import run_bass_kernel_spmd

F16 = mybir.dt.float16
F32 = mybir.dt.float32
AF = mybir.ActivationFunctionType

B, N, C, H, D = 2, 2048, 1024, 16, 64
NCORES = 8
SCALE = float(D) ** -0.5

TRACE = False
LAST_EXEC_NS = None
LAST_RESULTS = None

_COMPILED_NC = None


class _Emitter:
    def __init__(self, nc, tc, pools, tensors):
        self.nc = nc
        self.tc = tc
        self.p = pools
        self.t = tensors
        self.st = {}      # (b, h, m, qp) -> S_T tile
        self.ps_o = {}    # (b, qp) -> attnv psum accumulator
        self.ps_d = {}    # b -> denominator psum
        self.outTb = {}   # b -> normalized attn output tile
        self.n_st = 0

    # -- scores + exp for one (b, m): 8 matmuls, 4 exp regions ----------
    def scores(self, b, m):
        nc, t = self.nc, self.t
        ms = slice(b * 2048 + m * 128, b * 2048 + (m + 1) * 128)
        for qp in range(2):
            ps = [
                self.p["sc"].tile([128, 1024], F32, tag="sc", name=f"sc{b}{m}{qp}{h}")
                for h in range(2)
            ]
            for qc in range(2):
                cs = slice(qc * 512, (qc + 1) * 512)
                qs = slice(
                    b * 2048 + qp * 1024 + qc * 512,
                    b * 2048 + qp * 1024 + (qc + 1) * 512,
                )
                for h in range(2):
                    hp = slice(h * 64, (h + 1) * 64)
                    nc.tensor.matmul(
                        ps[h][:, cs],
                        lhsT=t["k2T"][hp, ms],
                        rhs=t["q2T"][hp, qs],
                        start=True,
                        stop=True,
                    )
            for h in range(2):
                st = self.p["st"].tile(
                    [128, 1024], F16, tag="st", name=f"st{b}{m}{qp}{h}"
                )
                nc.scalar.activation(st[:], ps[h][:], AF.Exp, scale=SCALE)
                self.st[(b, h, m, qp)] = st

    # -- attnv + denominators for one (b, qp, m) ------------------------
    def attnv(self, b, qp, m, opool="o"):
        nc, t = self.nc, self.t
        if m == 0:
            self.ps_o[(b, qp)] = self.p[opool].tile(
                [128, 1024], F32, tag=opool, name=f"o{b}{qp}"
            )
            if qp == 0:
                self.ps_d[b] = self.p["d"].tile(
                    [128, 1024], F32, tag="d", name=f"d{b}"
                )
        ps_o = self.ps_o[(b, qp)]
        ps_d = self.ps_d[b]
        kw = dict(start=(m == 0), stop=(m == 15))
        for qc in range(2):
            cs = slice(qc * 512, (qc + 1) * 512)
            for h in range(2):
                nc.tensor.matmul(
                    ps_o[h * 64 : (h + 1) * 64, cs],
                    lhsT=t["vpos"][
                        :,
                        (b * 2 + h) * 1024 + m * 64 : (b * 2 + h) * 1024 + (m + 1) * 64,
                    ],
                    rhs=self.st[(b, h, m, qp)][:, cs],
                    **kw,
                )
        # denominators: 4 concurrent M=1 column-tiled streams
        ds = slice(qp * 512, (qp + 1) * 512)
        for qc in range(2):
            cs = slice(qc * 512, (qc + 1) * 512)
            for h in range(2):
                row = h * 32 + qc * 64
                nc.tensor.matmul(
                    ps_d[row : row + 1, ds],
                    lhsT=t["ones"][:, 0:1],
                    rhs=self.st[(b, h, m, qp)][:, cs],
                    skip_group_check=True,
                    tile_position=(0, row),
                    **kw,
                )

    # -- normalize one (b, qp): outTb[:, qp] = ps_o / denom -------------
    def normalize(self, b, qp):
        nc = self.nc
        if b not in self.outTb:
            self.outTb[b] = self.p["ob"].tile(
                [128, 2048], F16, tag="outT", name=f"outT{b}"
            )
        outTb = self.outTb[b]
        ps_o = self.ps_o[(b, qp)]
        ps_d = self.ps_d[b]
        ds = slice(qp * 512, (qp + 1) * 512)
        rc = self.p["rc"].tile([128, 1024], F32, tag="rc", name=f"rc{b}{qp}")
        bcast = [0] * 32
        # denom rows: (h, qc) -> h*32 + qc*64; reciprocal in place, then
        # broadcast each row across its 32-partition quadrants:
        # rows 0-63 get 1/d_h0, rows 64-127 get 1/d_h1, per qc half.
        for qc in range(2):
            cr = slice(qc * 512, (qc + 1) * 512)
            for h in range(2):
                row = h * 32 + qc * 64
                nc.vector.reciprocal(rc[row : row + 1, cr], ps_d[row : row + 1, ds])
        for qc in range(2):
            cr = slice(qc * 512, (qc + 1) * 512)
            # sources: h0 at row qc*64, h1 at row 32+qc*64 (quadrant-first
            # rows).  Broadcast so rows 0-63 <- h0, rows 64-127 <- h1;
            # write non-source quadrants first, source quadrants last.
            if qc == 0:
                seq = [(64, 32), (96, 32), (32, 0), (0, 0)]
            else:
                seq = [(0, 64), (32, 64), (64, 96), (96, 96)]
            for dst, src in seq:
                nc.vector.stream_shuffle(
                    rc[dst : dst + 32, cr], rc[src : src + 32, cr], bcast
                )
            nc.vector.tensor_mul(
                outTb[:, qp * 1024 + qc * 512 : qp * 1024 + (qc + 1) * 512],
                ps_o[:, cr],
                rc[:, cr],
            )

    # -- output projection for one batch --------------------------------
    def proj(self, b):
        nc, t = self.nc, self.t
        outTb = self.outTb[b]
        for pm in range(16):
            ps_p = self.p["sc"].tile([128, 1024], F32, tag="sc", name=f"pp{b}{pm}")
            for ncol in range(2):
                nc.tensor.matmul(
                    ps_p[:, ncol * 512 : (ncol + 1) * 512],
                    lhsT=outTb[:, pm * 128 : (pm + 1) * 128],
                    rhs=t["wp_sb"][:, ncol * 512 : (ncol + 1) * 512],
                    start=True,
                    stop=True,
                )
            ev = self.p["pe"].tile([128, 1024], F32, tag="pe", name=f"pe{b}{pm}")
            nc.vector.tensor_copy(ev[:], ps_p[:])
            nc.sync.dma_start(t["outp"][b, pm * 128 : (pm + 1) * 128, :], ev[:])


def _emit(nc):
    xq = nc.dram_tensor("xq", [C, B * N], F16, kind="ExternalInput")
    xt = nc.dram_tensor("xt", [C, B * N], F16, kind="ExternalInput")
    wq = nc.dram_tensor("wq", [128, 1024], F16, kind="ExternalInput")
    wk = nc.dram_tensor("wk", [128, 1024], F16, kind="ExternalInput")
    wv = nc.dram_tensor("wv", [128, 1024], F16, kind="ExternalInput")
    wp = nc.dram_tensor("wp", [128, C], F16, kind="ExternalInput")
    id64 = nc.dram_tensor("id64", [128, 64], F16, kind="ExternalInput")
    outp = nc.dram_tensor("out_p", [B, N, C], F32, kind="ExternalOutput")

    with tile.TileContext(nc) as tc:
        with (
            tc.tile_pool(name="consts", bufs=1) as cpool,
            tc.tile_pool(name="xs", bufs=4) as xs,
            tc.tile_pool(name="big", bufs=1) as big,
            tc.tile_pool(name="stp", bufs=64) as stp,
            tc.tile_pool(name="ob", bufs=2) as obp,
            tc.tile_pool(name="rc", bufs=1) as rcpool,
            tc.tile_pool(name="pe", bufs=2) as pep,
            tc.tile_pool(name="sc", bufs=2, space="PSUM") as scp,
            tc.tile_pool(name="o", bufs=1, space="PSUM") as op,
            tc.tile_pool(name="d", bufs=1, space="PSUM") as dp,
        ):
            # ---- constants ------------------------------------------------
            w_sb = {}
            for name, dram in (("wq", wq), ("wk", wk), ("wv", wv)):
                t_ = cpool.tile([128, 1024], F16, tag=name, name=f"w_{name}")
                nc.sync.dma_start(t_[:], dram[:])
                w_sb[name] = t_
            wp_sb = cpool.tile([128, C], F16, tag="wp")
            nc.sync.dma_start(wp_sb[:], wp[:])
            id_sb = cpool.tile([128, 64], F16, tag="id64")
            nc.sync.dma_start(id_sb[:], id64[:])
            ones_sb = cpool.tile([128, 1], F16, tag="ones")
            nc.vector.memset(ones_sb[:], 1.0)

            q2T = big.tile([128, B * N], F16, tag="q2T")
            k2T = big.tile([128, B * N], F16, tag="k2T")
            v2T = big.tile([128, B * N], F16, tag="v2T")
            vpos = big.tile([128, B * 2 * N], F16, tag="vpos")

            # ---- Q/K/V projections (quarters of 1024 positions) -----------
            # q: one [128,1024] psum per quarter; kv share the xt stream.
            for qtr in range(4):
                qs = slice(qtr * 1024, (qtr + 1) * 1024)
                ps_q = scp.tile([128, 1024], F32, tag="sc", name=f"psq{qtr}")
                for kc in range(8):
                    x_t = xs.tile([128, 1024], F16, tag="x", name=f"xq{qtr}{kc}")
                    nc.sync.dma_start(x_t[:], xq[kc * 128 : (kc + 1) * 128, qs])
                    for qc in range(2):
                        cs = slice(qc * 512, (qc + 1) * 512)
                        nc.tensor.matmul(
                            ps_q[:, cs],
                            lhsT=w_sb["wq"][:, kc * 128 : (kc + 1) * 128],
                            rhs=x_t[:, cs],
                            start=(kc == 0),
                            stop=(kc == 7),
                        )
                nc.vector.tensor_copy(q2T[:, qs], ps_q[:])
            for qtr in range(4):
                qs = slice(qtr * 1024, (qtr + 1) * 1024)
                ps_k = scp.tile([128, 1024], F32, tag="sc", name=f"psk{qtr}")
                ps_v = op.tile([128, 1024], F32, tag="o", name=f"psv{qtr}")
                for kc in range(8):
                    x_t = xs.tile([128, 1024], F16, tag="x", name=f"xt{qtr}{kc}")
                    nc.sync.dma_start(x_t[:], xt[kc * 128 : (kc + 1) * 128, qs])
                    for qc in range(2):
                        cs = slice(qc * 512, (qc + 1) * 512)
                        nc.tensor.matmul(
                            ps_k[:, cs],
                            lhsT=w_sb["wk"][:, kc * 128 : (kc + 1) * 128],
                            rhs=x_t[:, cs],
                            start=(kc == 0),
                            stop=(kc == 7),
                        )
                        nc.tensor.matmul(
                            ps_v[:, cs],
                            lhsT=w_sb["wv"][:, kc * 128 : (kc + 1) * 128],
                            rhs=x_t[:, cs],
                            start=(kc == 0),
                            stop=(kc == 7),
                        )
                nc.vector.tensor_copy(k2T[:, qs], ps_k[:])
                nc.vector.tensor_copy(v2T[:, qs], ps_v[:])

            # ---- transpose v2T -> vpos (key-major) via PE -----------------
            for b in range(2):
                for h in range(2):
                    hp = slice(h * 64, (h + 1) * 64)
                    for oct_ in range(2):
                        ps_t = dp.tile([128, 512], F16, tag="d", name=f"pst{b}{h}{oct_}")
                        for i in range(8):
                            m = oct_ * 8 + i
                            ks = slice(b * 2048 + m * 128, b * 2048 + (m + 1) * 128)
                            nc.tensor.transpose(
                                ps_t[:, i * 64 : (i + 1) * 64],
                                v2T[hp, ks],
                                id_sb[hp, :],
                            )
                        nc.vector.tensor_copy(
                            vpos[
                                :,
                                (b * 2 + h) * 1024
                                + oct_ * 512 : (b * 2 + h) * 1024
                                + (oct_ + 1) * 512,
                            ],
                            ps_t[:],
                        )

            pools = {"sc": scp, "o": op, "d": dp, "st": stp, "ob": obp,
                     "rc": rcpool, "pe": pep}
            tensors = {"q2T": q2T, "k2T": k2T, "v2T": v2T, "vpos": vpos,
                       "wp_sb": wp_sb, "ones": ones_sb, "outp": outp}
            em = _Emitter(nc, tc, pools, tensors)

            # ---- software-pipelined attention -----------------------------
            # P1: scores(b0) | attnv(b0,qp0) at lag 1
            for m in range(16):
                em.scores(0, m)
                if m >= 1:
                    em.attnv(0, 0, m - 1)
            em.attnv(0, 0, 15)
            em.normalize(0, 0)
            # P2: scores(b1) | attnv(b0,qp1) at lag 1
            for m in range(16):
                em.scores(1, m)
                if m >= 1:
                    em.attnv(0, 1, m - 1)
            em.attnv(0, 1, 15)
            em.normalize(0, 1)
            em.proj(0)
            # P3: attnv(b1), both qp passes interleaved (qp1 borrows an
            # sc-pool psum slot so the passes run concurrently)
            for m in range(16):
                em.attnv(1, 0, m)
                em.attnv(1, 1, m, opool="sc")
            em.normalize(1, 0)
            em.normalize(1, 1)
            em.proj(1)
    return nc


def _get_compiled():
    global _COMPILED_NC
    if _COMPILED_NC is None:
        nc = bacc.Bacc(
            "TRN2", target_bir_lowering=False, debug=False, num_devices=NCORES
        )
        _emit(nc)
        nc.compile()
        _COMPILED_NC = nc
    return _COMPILED_NC


def _install_trace_shim():
    """Register antenv.axon_hooks NTFF hook (missing on this image)."""
    import contextlib
    import ctypes
    import types

    if "antenv.axon_hooks" in sys.modules:
        return
    try:
        import antenv
    except ImportError:
        return
    so_path = "/opt/axon/libaxon_pjrt.so"
    if not os.path.exists(so_path):
        return

    mod = types.ModuleType("antenv.axon_hooks")
    mod._hook = None
    mod.set_axon_ntff_profile_hook = lambda h: setattr(mod, "_hook", h)
    mod.get_axon_ntff_profile_hook = lambda: mod._hook

    lib = ctypes.CDLL(so_path)
    if not hasattr(lib, "axon_start_nrt_profile"):
        return
    lib.axon_start_nrt_profile.argtypes = [
        ctypes.POINTER(ctypes.c_int64),
        ctypes.c_size_t,
    ]
    lib.axon_start_nrt_profile.restype = ctypes.c_int64
    lib.axon_stop_nrt_profile.argtypes = [ctypes.c_char_p]
    lib.axon_stop_nrt_profile.restype = ctypes.c_int64

    @contextlib.contextmanager
    def _hook(output_dir, device_ids):
        import jax

        jax.devices()
        if device_ids:
            ids = (ctypes.c_int64 * len(device_ids))(*device_ids)
            rc = lib.axon_start_nrt_profile(ids, len(device_ids))
        else:
            rc = lib.axon_start_nrt_profile(None, 0)
        if rc != 0:
            raise RuntimeError(f"axon_start_nrt_profile rc={rc}")
        try:
            yield
        finally:
            n = lib.axon_stop_nrt_profile(str(output_dir).encode())
            if n < 0:
                raise RuntimeError(f"axon_stop_nrt_profile rc={n}")

    mod.set_axon_ntff_profile_hook(_hook)
    sys.modules["antenv.axon_hooks"] = mod
    antenv.axon_hooks = mod


def kernel(query, target, Wq, Wk, Wv, Wp, bp):
    global LAST_EXEC_NS, LAST_RESULTS
    query = np.asarray(query, dtype=np.float32)
    target = np.asarray(target, dtype=np.float32)
    Wq = np.asarray(Wq, dtype=np.float32)
    Wk = np.asarray(Wk, dtype=np.float32)
    Wv = np.asarray(Wv, dtype=np.float32)
    Wp = np.asarray(Wp, dtype=np.float32)
    bp = np.asarray(bp, dtype=np.float32)

    xq = np.ascontiguousarray(query.reshape(B * N, C).T).astype(np.float16)
    xt = np.ascontiguousarray(target.reshape(B * N, C).T).astype(np.float16)
    id64 = np.zeros((128, 64), dtype=np.float16)
    for p in range(128):
        id64[p, p % 64] = 1.0

    def wlayout(Wm, rows):
        # SBUF weight tile [p, kc*128 + m] = W[row0 + m, kc*128 + p]
        ws = Wm[rows, :].astype(np.float16)  # (128, 1024)
        return np.ascontiguousarray(
            ws.reshape(128, 8, 128).transpose(2, 1, 0).reshape(128, 1024)
        )

    in_maps = []
    for c in range(NCORES):
        rows = slice(c * 128, (c + 1) * 128)
        in_maps.append(
            {
                "xq": xq,
                "xt": xt,
                "wq": wlayout(Wq, rows),
                "wk": wlayout(Wk, rows),
                "wv": wlayout(Wv, rows),
                "wp": np.ascontiguousarray(Wp[:, rows].T).astype(np.float16),
                "id64": id64,
            }
        )

    if TRACE:
        _install_trace_shim()

    nc = _get_compiled()
    res = run_bass_kernel_spmd(
        nc, in_maps, core_ids=list(range(NCORES)), trace=TRACE
    )
    LAST_RESULTS = res
    LAST_EXEC_NS = res.exec_time_ns

    acc = res.results[0]["out_p"].astype(np.float64)
    for c in range(1, NCORES):
        acc += res.results[c]["out_p"]
    out = acc.astype(np.float32) + bp[None, None, :]
    return out


# revision 15
# speedup vs baseline: 1.4545x; 1.2290x over previous
"""Trainium2 Bass kernel for nn_CrossAttention (B=2, N=2048, C=1024, H=16).

Sharding: 16 heads / 8 cores = 2 heads per core (both batches on every
core).  Each core computes its heads' Q/K/V projections with the matching
128-row slice of Wq/Wk/Wv, full attention for its 4 (batch, head) pairs,
and a partial output projection against its 128-column slice of Wp.  The
host sums the 8 partial projections (the tensor-parallel all-reduce) and
adds the bias.

Schedule (per core): the exp of 16.8M score elements on the Scalar engine
(~134us) is the critical resource, so scores->exp for batch b+/-0 is
software-pipelined against attnv/denominator matmuls at lag-1 so the
Scalar engine never starves and the PE never idles long enough for HAM to
re-throttle its clock.

PSUM budget (8 banks of 2KB/partition):
  sc pool   2 x [128,1024] f32 = 4 banks  (score regions / qkv / proj)
  o  pool   1 x [128,1024] f32 = 2 banks  (attnv accumulator per qp pass)
  d  pool   1 x [128,1024] f32 = 2 banks  (denominator rows, packed)

On-device layouts (per core, fp16 matmul operands / fp32 PSUM):
  xq/xt   [1024 ch, 4096 pos]   channel-major inputs (host pre-transposed)
  q2T/k2T/v2T [128, 4096]       rows 0-63 head0 dims, 64-127 head1 dims
  vpos    [128 keys, 4x16x64]   v transposed back to key-major via PE
  S_T     [128 keys, 1024 q]    exp(scores^T) tile per (h, m, qp), fp16
  outTb   [128 dims, 2048 q]    normalized attention output, per batch
  out_p   [2, 2048, 1024] f32   partial projection (summed on host)
"""

import os
import sys

for _p in ("/opt/trn_rl_repo", os.path.expanduser("~/.axon_site/_ro/trn_rl_repo")):
    if os.path.isdir(_p) and _p not in sys.path:
        sys.path.insert(0, _p)

import numpy as np

import concourse.bacc as bacc
import concourse.mybir as mybir
import concourse.tile as tile
from concourse.bass_utils import run_bass_kernel_spmd

F16 = mybir.dt.float16
F32 = mybir.dt.float32
AF = mybir.ActivationFunctionType

B, N, C, H, D = 2, 2048, 1024, 16, 64
NCORES = 8
SCALE = float(D) ** -0.5

TRACE = False
LAST_EXEC_NS = None
LAST_RESULTS = None

_COMPILED_NC = None


class _Emitter:
    def __init__(self, nc, tc, pools, tensors):
        self.nc = nc
        self.tc = tc
        self.p = pools
        self.t = tensors
        self.st = {}      # (b, h, m, qp) -> S_T tile
        self.ps_o = {}    # (b, qp) -> attnv psum accumulator
        self.ps_d = {}    # b -> denominator psum
        self.outTb = {}   # b -> normalized attn output tile
        self.n_st = 0

    # -- scores + exp for one (b, m, qp): 4 matmuls, 2 exp regions ------
    def scores(self, b, m, qp):
        nc, t = self.nc, self.t
        ms = slice(b * 2048 + m * 128, b * 2048 + (m + 1) * 128)
        ps = [
            self.p["sc"].tile([128, 1024], F32, tag="sc", name=f"sc{b}{m}{qp}{h}")
            for h in range(2)
        ]
        for qc in range(2):
            cs = slice(qc * 512, (qc + 1) * 512)
            qs = slice(
                b * 2048 + qp * 1024 + qc * 512,
                b * 2048 + qp * 1024 + (qc + 1) * 512,
            )
            for h in range(2):
                hp = slice(h * 64, (h + 1) * 64)
                nc.tensor.matmul(
                    ps[h][:, cs],
                    lhsT=t["k2T"][hp, ms],
                    rhs=t["q2T"][hp, qs],
                    start=True,
                    stop=True,
                )
        for h in range(2):
            st = self.p["st"].tile(
                [128, 1024], F16, tag="st", name=f"st{b}{m}{qp}{h}"
            )
            nc.scalar.activation(st[:], ps[h][:], AF.Exp, scale=SCALE)
            self.st[(b, h, m, qp)] = st

    # -- attnv + denominators for one (b, qp, m) ------------------------
    def attnv(self, b, qp, m, opool="o"):
        nc, t = self.nc, self.t
        if m == 0:
            self.ps_o[(b, qp)] = self.p[opool].tile(
                [128, 1024], F32, tag=opool, name=f"o{b}{qp}"
            )
            if qp == 0:
                self.ps_d[b] = self.p["d"].tile(
                    [128, 1024], F32, tag="d", name=f"d{b}"
                )
        ps_o = self.ps_o[(b, qp)]
        ps_d = self.ps_d[b]
        kw = dict(start=(m == 0), stop=(m == 15))
        for qc in range(2):
            cs = slice(qc * 512, (qc + 1) * 512)
            for h in range(2):
                nc.tensor.matmul(
                    ps_o[h * 64 : (h + 1) * 64, cs],
                    lhsT=t["vpos"][
                        :,
                        (b * 2 + h) * 1024 + m * 64 : (b * 2 + h) * 1024 + (m + 1) * 64,
                    ],
                    rhs=self.st[(b, h, m, qp)][:, cs],
                    **kw,
                )
        # denominators: 4 concurrent M=1 column-tiled streams
        ds = slice(qp * 512, (qp + 1) * 512)
        for qc in range(2):
            cs = slice(qc * 512, (qc + 1) * 512)
            for h in range(2):
                row = h * 32 + qc * 64
                nc.tensor.matmul(
                    ps_d[row : row + 1, ds],
                    lhsT=t["ones"][:, 0:1],
                    rhs=self.st[(b, h, m, qp)][:, cs],
                    skip_group_check=True,
                    tile_position=(0, row),
                    **kw,
                )

    # -- normalize one (b, qp): outTb[:, qp] = ps_o / denom -------------
    def normalize(self, b, qp):
        nc = self.nc
        if b not in self.outTb:
            self.outTb[b] = self.p["ob"].tile(
                [128, 2048], F16, tag="outT", name=f"outT{b}"
            )
        outTb = self.outTb[b]
        ps_o = self.ps_o[(b, qp)]
        ps_d = self.ps_d[b]
        ds = slice(qp * 512, (qp + 1) * 512)
        rc = self.p["rc"].tile([128, 1024], F32, tag="rc", name=f"rc{b}{qp}")
        bcast = [0] * 32
        # denom rows: (h, qc) -> h*32 + qc*64; reciprocal in place, then
        # broadcast each row across its 32-partition quadrants:
        # rows 0-63 get 1/d_h0, rows 64-127 get 1/d_h1, per qc half.
        for qc in range(2):
            cr = slice(qc * 512, (qc + 1) * 512)
            for h in range(2):
                row = h * 32 + qc * 64
                nc.vector.reciprocal(rc[row : row + 1, cr], ps_d[row : row + 1, ds])
        for qc in range(2):
            cr = slice(qc * 512, (qc + 1) * 512)
            # sources: h0 at row qc*64, h1 at row 32+qc*64 (quadrant-first
            # rows).  Broadcast so rows 0-63 <- h0, rows 64-127 <- h1;
            # write non-source quadrants first, source quadrants last.
            if qc == 0:
                seq = [(64, 32), (96, 32), (32, 0), (0, 0)]
            else:
                seq = [(0, 64), (32, 64), (64, 96), (96, 96)]
            for dst, src in seq:
                nc.vector.stream_shuffle(
                    rc[dst : dst + 32, cr], rc[src : src + 32, cr], bcast
                )
            nc.vector.tensor_mul(
                outTb[:, qp * 1024 + qc * 512 : qp * 1024 + (qc + 1) * 512],
                ps_o[:, cr],
                rc[:, cr],
            )

    # -- output projection for one batch --------------------------------
    def proj(self, b):
        nc, t = self.nc, self.t
        outTb = self.outTb[b]
        for pm in range(16):
            ps_p = self.p["sc"].tile([128, 1024], F32, tag="sc", name=f"pp{b}{pm}")
            for ncol in range(2):
                nc.tensor.matmul(
                    ps_p[:, ncol * 512 : (ncol + 1) * 512],
                    lhsT=outTb[:, pm * 128 : (pm + 1) * 128],
                    rhs=t["wp_sb"][:, ncol * 512 : (ncol + 1) * 512],
                    start=True,
                    stop=True,
                )
            ev = self.p["pe"].tile([128, 1024], F32, tag="pe", name=f"pe{b}{pm}")
            nc.vector.tensor_copy(ev[:], ps_p[:])
            nc.sync.dma_start(t["outp"][b, pm * 128 : (pm + 1) * 128, :], ev[:])


def _emit(nc):
    xq = nc.dram_tensor("xq", [C, B * N], F16, kind="ExternalInput")
    xt = nc.dram_tensor("xt", [C, B * N], F16, kind="ExternalInput")
    wq = nc.dram_tensor("wq", [128, 1024], F16, kind="ExternalInput")
    wk = nc.dram_tensor("wk", [128, 1024], F16, kind="ExternalInput")
    wv = nc.dram_tensor("wv", [128, 1024], F16, kind="ExternalInput")
    wp = nc.dram_tensor("wp", [128, C], F16, kind="ExternalInput")
    id64 = nc.dram_tensor("id64", [128, 64], F16, kind="ExternalInput")
    outp = nc.dram_tensor("out_p", [B, N, C], F32, kind="ExternalOutput")

    with tile.TileContext(nc) as tc:
        with (
            tc.tile_pool(name="consts", bufs=1) as cpool,
            tc.tile_pool(name="xs", bufs=6) as xs,
            tc.tile_pool(name="big", bufs=1) as big,
            tc.tile_pool(name="stp", bufs=40) as stp,
            tc.tile_pool(name="ob", bufs=2) as obp,
            tc.tile_pool(name="rc", bufs=2) as rcpool,
            tc.tile_pool(name="pe", bufs=3) as pep,
            tc.tile_pool(name="sc", bufs=2, space="PSUM") as scp,
            tc.tile_pool(name="o", bufs=1, space="PSUM") as op,
            tc.tile_pool(name="d", bufs=1, space="PSUM") as dp,
        ):
            # ---- constants ------------------------------------------------
            w_sb = {}
            for name, dram in (("wq", wq), ("wk", wk), ("wv", wv)):
                t_ = cpool.tile([128, 1024], F16, tag=name, name=f"w_{name}")
                nc.sync.dma_start(t_[:], dram[:])
                w_sb[name] = t_
            wp_sb = cpool.tile([128, C], F16, tag="wp")
            nc.sync.dma_start(wp_sb[:], wp[:])
            id_sb = cpool.tile([128, 64], F16, tag="id64")
            nc.sync.dma_start(id_sb[:], id64[:])
            ones_sb = cpool.tile([128, 1], F16, tag="ones")
            nc.vector.memset(ones_sb[:], 1.0)

            q2T = big.tile([128, B * N], F16, tag="q2T")
            k2T = big.tile([128, B * N], F16, tag="k2T")
            v2T = big.tile([128, B * N], F16, tag="v2T")
            vpos = big.tile([128, B * 2 * N], F16, tag="vpos")

            # ---- Q/K/V projection helpers (quarters of 1024 positions) ----
            def q_quarter(qtr, pool, ptag):
                qs = slice(qtr * 1024, (qtr + 1) * 1024)
                ps_q = pool.tile([128, 1024], F32, tag=ptag, name=f"psq{qtr}")
                for kc in range(8):
                    x_t = xs.tile([128, 1024], F16, tag="x", name=f"xq{qtr}{kc}")
                    nc.sync.dma_start(x_t[:], xq[kc * 128 : (kc + 1) * 128, qs])
                    for qc in range(2):
                        cs = slice(qc * 512, (qc + 1) * 512)
                        nc.tensor.matmul(
                            ps_q[:, cs],
                            lhsT=w_sb["wq"][:, kc * 128 : (kc + 1) * 128],
                            rhs=x_t[:, cs],
                            start=(kc == 0),
                            stop=(kc == 7),
                        )
                    if kc % 2 == 1:
                        yield
                nc.vector.tensor_copy(q2T[:, qs], ps_q[:])

            def kv_quarter(qtr, poolk, ktag, poolv, vtag):
                qs = slice(qtr * 1024, (qtr + 1) * 1024)
                ps_k = poolk.tile([128, 1024], F32, tag=ktag, name=f"psk{qtr}")
                ps_v = poolv.tile([128, 1024], F32, tag=vtag, name=f"psv{qtr}")
                for kc in range(8):
                    x_t = xs.tile([128, 1024], F16, tag="x", name=f"xt{qtr}{kc}")
                    nc.sync.dma_start(x_t[:], xt[kc * 128 : (kc + 1) * 128, qs])
                    for qc in range(2):
                        cs = slice(qc * 512, (qc + 1) * 512)
                        nc.tensor.matmul(
                            ps_k[:, cs],
                            lhsT=w_sb["wk"][:, kc * 128 : (kc + 1) * 128],
                            rhs=x_t[:, cs],
                            start=(kc == 0),
                            stop=(kc == 7),
                        )
                        nc.tensor.matmul(
                            ps_v[:, cs],
                            lhsT=w_sb["wv"][:, kc * 128 : (kc + 1) * 128],
                            rhs=x_t[:, cs],
                            start=(kc == 0),
                            stop=(kc == 7),
                        )
                    if kc % 2 == 1:
                        yield
                nc.vector.tensor_copy(k2T[:, qs], ps_k[:])
                nc.vector.tensor_copy(v2T[:, qs], ps_v[:])

            def transposes(b):
                for h in range(2):
                    hp = slice(h * 64, (h + 1) * 64)
                    for oct_ in range(2):
                        ps_t = dp.tile(
                            [128, 512], F16, tag="d", name=f"pst{b}{h}{oct_}"
                        )
                        for i in range(8):
                            m = oct_ * 8 + i
                            ks = slice(b * 2048 + m * 128, b * 2048 + (m + 1) * 128)
                            nc.tensor.transpose(
                                ps_t[:, i * 64 : (i + 1) * 64],
                                v2T[hp, ks],
                                id_sb[hp, :],
                            )
                        nc.vector.tensor_copy(
                            vpos[
                                :,
                                (b * 2 + h) * 1024
                                + oct_ * 512 : (b * 2 + h) * 1024
                                + (oct_ + 1) * 512,
                            ],
                            ps_t[:],
                        )

            def drain(gen):
                for _ in gen:
                    pass

            # ---- S0: batch-0 qkv + v-transposes ---------------------------
            drain(q_quarter(0, scp, "sc"))
            drain(q_quarter(1, scp, "sc"))
            drain(kv_quarter(0, scp, "sc", op, "o"))
            drain(kv_quarter(1, scp, "sc", op, "o"))
            transposes(0)

            pools = {"sc": scp, "o": op, "d": dp, "st": stp, "ob": obp,
                     "rc": rcpool, "pe": pep}
            tensors = {"q2T": q2T, "k2T": k2T, "v2T": v2T, "vpos": vpos,
                       "wp_sb": wp_sb, "ones": ones_sb, "outp": outp}
            em = _Emitter(nc, tc, pools, tensors)

            # ---- U0: scores(b0,qp0) woven with batch-1 qkv ----------------
            def b1_qkv_gen():
                yield from q_quarter(2, op, "o")
                yield from q_quarter(3, op, "o")
                yield from kv_quarter(2, op, "o", dp, "d")
                yield from kv_quarter(3, op, "o", dp, "d")

            gen = b1_qkv_gen()
            for m in range(16):
                em.scores(0, m, 0)
                next(gen, None)
            drain(gen)
            # ---- U1: attnv(b0,qp0) | scores(b0,qp1) -----------------------
            transposes(1)
            for m in range(16):
                em.attnv(0, 0, m)
                em.scores(0, m, 1)
            em.normalize(0, 0)
            # ---- U2: attnv(b0,qp1) | scores(b1,qp0) -----------------------
            for m in range(16):
                em.attnv(0, 1, m)
                em.scores(1, m, 0)
            em.normalize(0, 1)
            # ---- U3: attnv(b1,qp0) | scores(b1,qp1) -----------------------
            for m in range(16):
                em.attnv(1, 0, m)
                em.scores(1, m, 1)
            em.normalize(1, 0)
            em.proj(0)
            # ---- U4: attnv(b1,qp1) tail -----------------------------------
            for m in range(16):
                em.attnv(1, 1, m)
            em.normalize(1, 1)
            em.proj(1)
    return nc


def _get_compiled():
    global _COMPILED_NC
    if _COMPILED_NC is None:
        nc = bacc.Bacc(
            "TRN2", target_bir_lowering=False, debug=False, num_devices=NCORES
        )
        _emit(nc)
        nc.compile()
        _COMPILED_NC = nc
    return _COMPILED_NC


def _install_trace_shim():
    """Register antenv.axon_hooks NTFF hook (missing on this image)."""
    import contextlib
    import ctypes
    import types

    if "antenv.axon_hooks" in sys.modules:
        return
    try:
        import antenv
    except ImportError:
        return
    so_path = "/opt/axon/libaxon_pjrt.so"
    if not os.path.exists(so_path):
        return

    mod = types.ModuleType("antenv.axon_hooks")
    mod._hook = None
    mod.set_axon_ntff_profile_hook = lambda h: setattr(mod, "_hook", h)
    mod.get_axon_ntff_profile_hook = lambda: mod._hook

    lib = ctypes.CDLL(so_path)
    if not hasattr(lib, "axon_start_nrt_profile"):
        return
    lib.axon_start_nrt_profile.argtypes = [
        ctypes.POINTER(ctypes.c_int64),
        ctypes.c_size_t,
    ]
    lib.axon_start_nrt_profile.restype = ctypes.c_int64
    lib.axon_stop_nrt_profile.argtypes = [ctypes.c_char_p]
    lib.axon_stop_nrt_profile.restype = ctypes.c_int64

    @contextlib.contextmanager
    def _hook(output_dir, device_ids):
        import jax

        jax.devices()
        if device_ids:
            ids = (ctypes.c_int64 * len(device_ids))(*device_ids)
            rc = lib.axon_start_nrt_profile(ids, len(device_ids))
        else:
            rc = lib.axon_start_nrt_profile(None, 0)
        if rc != 0:
            raise RuntimeError(f"axon_start_nrt_profile rc={rc}")
        try:
            yield
        finally:
            n = lib.axon_stop_nrt_profile(str(output_dir).encode())
            if n < 0:
                raise RuntimeError(f"axon_stop_nrt_profile rc={n}")

    mod.set_axon_ntff_profile_hook(_hook)
    sys.modules["antenv.axon_hooks"] = mod
    antenv.axon_hooks = mod


def kernel(query, target, Wq, Wk, Wv, Wp, bp):
    global LAST_EXEC_NS, LAST_RESULTS
    query = np.asarray(query, dtype=np.float32)
    target = np.asarray(target, dtype=np.float32)
    Wq = np.asarray(Wq, dtype=np.float32)
    Wk = np.asarray(Wk, dtype=np.float32)
    Wv = np.asarray(Wv, dtype=np.float32)
    Wp = np.asarray(Wp, dtype=np.float32)
    bp = np.asarray(bp, dtype=np.float32)

    xq = np.ascontiguousarray(query.reshape(B * N, C).T).astype(np.float16)
    xt = np.ascontiguousarray(target.reshape(B * N, C).T).astype(np.float16)
    id64 = np.zeros((128, 64), dtype=np.float16)
    for p in range(128):
        id64[p, p % 64] = 1.0

    def wlayout(Wm, rows):
        # SBUF weight tile [p, kc*128 + m] = W[row0 + m, kc*128 + p]
        ws = Wm[rows, :].astype(np.float16)  # (128, 1024)
        return np.ascontiguousarray(
            ws.reshape(128, 8, 128).transpose(2, 1, 0).reshape(128, 1024)
        )

    in_maps = []
    for c in range(NCORES):
        rows = slice(c * 128, (c + 1) * 128)
        in_maps.append(
            {
                "xq": xq,
                "xt": xt,
                "wq": wlayout(Wq, rows),
                "wk": wlayout(Wk, rows),
                "wv": wlayout(Wv, rows),
                "wp": np.ascontiguousarray(Wp[:, rows].T).astype(np.float16),
                "id64": id64,
            }
        )

    if TRACE:
        _install_trace_shim()

    nc = _get_compiled()
    res = run_bass_kernel_spmd(
        nc, in_maps, core_ids=list(range(NCORES)), trace=TRACE
    )
    LAST_RESULTS = res
    LAST_EXEC_NS = res.exec_time_ns

    acc = res.results[0]["out_p"].astype(np.float64)
    for c in range(1, NCORES):
        acc += res.results[c]["out_p"]
    out = acc.astype(np.float32) + bp[None, None, :]
    return out
